# revision 8
# baseline (speedup 1.0000x reference)
"""EnsembleActor MLP kernel for Trainium2 (Bass/Tile), expert-parallel over 8 cores.

Math per ensemble head e (E=8, B=4096, OBS=256, H=1024, A=64):
    h1 = relu(x @ W1 + b1)
    h2 = relu(h1 @ W2 + b2)
    mu = h2 @ W3 + b3
    Gs = sum(|mu|, axis=-1)/A ; g = max(Gs, 1)
    mu = mu / g ; pi = mu + 0.1*noise
    return tanh(mu), tanh(pi)

Sharding: one head per NeuronCore (8 heads, 8 cores). Each core runs the same
Bass program; per-core inputs differ.

Device layout strategy: activations flow feature-major ([feat, batch]) through
layers 1-2 so weights are the PE-stationary operand in natural [K, M] layout
(minimal LDWEIGHTS traffic), then layer 3 flips to batch-major ([batch, A])
by making h2 slices the stationary operand, which makes the per-row epilogue
(abs-sum over A, clamp, tanh) pure free-dim work for DVE/ACT.
Matmuls run as float32r (full PE rate at moving-dim>=256, fp32 storage) for
layers 1-2; layer 3 uses bf16 h2/W3 (FWL halves its LDWEIGHTS cost, which
dominates there). b3 is added via a K=1 ones-matmul into the same PSUM group.
"""

import os
import sys

import numpy as np

for _p in ("/opt/trn_rl_repo", os.path.expanduser("~/.axon_site/_ro/trn_rl_repo")):
    if os.path.isdir(_p) and _p not in sys.path:
        sys.path.insert(0, _p)

E, B, OBS, H, A = 8, 4096, 256, 1024, 64
ACT_NOISE = 0.1
P = 128          # SBUF/PSUM partitions
BT = 512         # batch tile (matmul moving free dim; one PSUM bank fp32)
NBT = B // BT    # 8 batch tiles
KO = OBS // P    # 2 k-chunks in layer 1
KH = H // P      # 8 k-chunks in layers 2/3
NBB = BT // P    # 4 batch sub-tiles of 128 rows inside a batch tile

_PROGRAM = None  # (nc, input_names) cache — build/compile once per process


def _build_program():
    from contextlib import ExitStack

    import concourse.bass as bass
    import concourse.tile as tile
    from concourse import bacc, mybir

    f32 = mybir.dt.float32
    f32r = mybir.dt.float32r
    bf16 = mybir.dt.bfloat16
    FT = mybir.ActivationFunctionType

    nc = bacc.Bacc("TRN2", target_bir_lowering=False, debug=False)

    xT = nc.dram_tensor("xT", [OBS, B], f32r, kind="ExternalInput").ap()
    noise = nc.dram_tensor("noise01", [B, A], f32, kind="ExternalInput").ap()
    W1 = nc.dram_tensor("W1", [OBS, H], f32r, kind="ExternalInput").ap()
    W2 = nc.dram_tensor("W2", [H, H], f32r, kind="ExternalInput").ap()
    W3 = nc.dram_tensor("W3bf", [H, A], bf16, kind="ExternalInput").ap()
    b1 = nc.dram_tensor("b1c", [P, H // P], f32, kind="ExternalInput").ap()
    b2 = nc.dram_tensor("b2c", [P, H // P], f32, kind="ExternalInput").ap()
    b3 = nc.dram_tensor("b3r", [1, A], f32r, kind="ExternalInput").ap()
    ones_in = nc.dram_tensor("ones_r", [1, P], f32r, kind="ExternalInput").ap()
    mu_out = nc.dram_tensor("mu", [B, A], f32, kind="ExternalOutput").ap()
    pi_out = nc.dram_tensor("pi", [B, A], f32, kind="ExternalOutput").ap()

    # [B, A] viewed as [NBT, P, NBB, A]: partition dim = row-within-subtile
    noise_v = noise.rearrange("(t b p) a -> t p b a", t=NBT, b=NBB, p=P)
    mu_v = mu_out.rearrange("(t b p) a -> t p b a", t=NBT, b=NBB, p=P)
    pi_v = pi_out.rearrange("(t b p) a -> t p b a", t=NBT, b=NBB, p=P)

    with tile.TileContext(nc) as tc, ExitStack() as ctx:
        wpool = ctx.enter_context(tc.tile_pool(name="weights", bufs=1))
        xpool = ctx.enter_context(tc.tile_pool(name="x", bufs=2))
        hpool = ctx.enter_context(tc.tile_pool(name="h", bufs=2))
        iopool = ctx.enter_context(tc.tile_pool(name="io", bufs=2))
        spool = ctx.enter_context(tc.tile_pool(name="small", bufs=4))
        pspool = ctx.enter_context(tc.tile_pool(name="ps", bufs=3, space="PSUM"))
        ps3pool = ctx.enter_context(tc.tile_pool(name="ps3", bufs=4, space="PSUM"))

        # ---- persistent weights/constants in SBUF ----
        w1s = []
        for k in range(KO):
            t = wpool.tile([P, H], f32r, name=f"w1_{k}", tag=f"w1_{k}")
            nc.sync.dma_start(out=t[:], in_=W1[k * P:(k + 1) * P, :])
            w1s.append(t)
        w2s = []
        for k in range(KH):
            t = wpool.tile([P, H], f32r, name=f"w2_{k}", tag=f"w2_{k}")
            nc.sync.dma_start(out=t[:], in_=W2[k * P:(k + 1) * P, :])
            w2s.append(t)
        w3s = wpool.tile([P, KH, A], bf16, name="w3s", tag="w3s")
        nc.sync.dma_start(
            out=w3s[:], in_=W3.rearrange("(k p) a -> p k a", k=KH, p=P)
        )
        b1s = wpool.tile([P, H // P], f32, name="b1s", tag="b1s")
        nc.sync.dma_start(out=b1s[:], in_=b1[:, :])
        b2s = wpool.tile([P, H // P], f32, name="b2s", tag="b2s")
        nc.sync.dma_start(out=b2s[:], in_=b2[:, :])
        b3s = wpool.tile([1, A], f32r, name="b3s", tag="b3s")
        nc.sync.dma_start(out=b3s[:], in_=b3[:, :])
        ones = wpool.tile([1, P], f32r, name="ones", tag="ones")
        nc.sync.dma_start(out=ones[:], in_=ones_in[:, :])

        for bt in range(NBT):
            bsl = bass.ds(bt * BT, BT)

            xts = []
            for k in range(KO):
                t = xpool.tile([P, BT], f32r, name=f"xt{k}", tag=f"xt{k}")
                nc.sync.dma_start(out=t[:], in_=xT[k * P:(k + 1) * P, bsl])
                xts.append(t)
            nz = iopool.tile([P, NBB, A], f32, name="nz", tag="nz")
            nc.sync.dma_start(out=nz[:], in_=noise_v[bt])

            # ---- layer 1: h1[o, b] = relu(sum_k W1[k, o]^T x[k, b] + b1[o]) ----
            h1s = []
            for oc in range(KH):
                ps = pspool.tile([P, BT], f32, name="ps1", tag="ps")
                for k in range(KO):
                    nc.tensor.matmul(
                        ps[:],
                        lhsT=w1s[k][:, oc * P:(oc + 1) * P],
                        rhs=xts[k][:],
                        start=(k == 0),
                        stop=(k == KO - 1),
                    )
                h = hpool.tile([P, BT], f32r, name=f"h1_{oc}", tag=f"h1_{oc}")
                nc.vector.tensor_scalar(
                    out=h[:], in0=ps[:],
                    scalar1=b1s[:, oc:oc + 1], scalar2=0.0,
                    op0=mybir.AluOpType.add, op1=mybir.AluOpType.max,
                )
                h1s.append(h)

            # ---- layer 2: h2[o, b] = relu(sum_k W2[k, o]^T h1[k, b] + b2[o]) ----
            # h2 stored bf16: it is layer 3's stationary operand (FWL needs !=fp32)
            h2s = []
            for oc in range(KH):
                ps = pspool.tile([P, BT], f32, name="ps2", tag="ps")
                for k in range(KH):
                    nc.tensor.matmul(
                        ps[:],
                        lhsT=w2s[k][:, oc * P:(oc + 1) * P],
                        rhs=h1s[k][:],
                        start=(k == 0),
                        stop=(k == KH - 1),
                    )
                h = hpool.tile([P, BT], bf16, name=f"h2_{oc}", tag=f"h2_{oc}")
                nc.vector.tensor_scalar(
                    out=h[:], in0=ps[:],
                    scalar1=b2s[:, oc:oc + 1], scalar2=0.0,
                    op0=mybir.AluOpType.add, op1=mybir.AluOpType.max,
                )
                h2s.append(h)

            # ---- layer 3 + epilogue, per 128-row sub-tile ----
            mus = iopool.tile([P, NBB, A], f32, name="mus", tag="mus")
            pis = iopool.tile([P, NBB, A], f32, name="pis", tag="pis")
            for bb in range(NBB):
                ps = ps3pool.tile([P, A], f32, name="ps3", tag="ps3")
                for k in range(KH):
                    nc.tensor.matmul(
                        ps[:],
                        lhsT=h2s[k][:, bb * P:(bb + 1) * P],
                        rhs=w3s[:, k, :],
                        start=(k == 0),
                        stop=False,
                    )
                nc.tensor.matmul(
                    ps[:], lhsT=ones[:], rhs=b3s[:], start=False, stop=True,
                )
                # Gs clamp + reciprocal
                gs = spool.tile([P, 1], f32, name="gs", tag="gs")
                nc.vector.tensor_reduce(
                    out=gs[:], in_=ps[:], axis=mybir.AxisListType.X,
                    op=mybir.AluOpType.add, apply_absolute_value=True,
                )
                g = spool.tile([P, 1], f32, name="g", tag="g")
                nc.vector.tensor_scalar(
                    out=g[:], in0=gs[:], scalar1=1.0 / A, scalar2=1.0,
                    op0=mybir.AluOpType.mult, op1=mybir.AluOpType.max,
                )
                rcp = spool.tile([P, 1], f32, name="rcp", tag="rcp")
                nc.vector.reciprocal(out=rcp[:], in_=g[:])
                # mu = tanh(ps * rcp)
                nc.scalar.activation(
                    out=mus[:, bb, :], in_=ps[:], func=FT.Tanh,
                    scale=rcp[:, 0:1],
                )
                # pi = tanh(ps * rcp + 0.1*noise)   (noise pre-scaled on host)
                pp = spool.tile([P, A], f32, name="pp", tag="pp")
                nc.vector.scalar_tensor_tensor(
                    out=pp[:], in0=ps[:], scalar=rcp[:, 0:1], in1=nz[:, bb, :],
                    op0=mybir.AluOpType.mult, op1=mybir.AluOpType.add,
                )
                nc.scalar.activation(out=pis[:, bb, :], in_=pp[:], func=FT.Tanh)
            nc.sync.dma_start(out=mu_v[bt], in_=mus[:])
            nc.sync.dma_start(out=pi_v[bt], in_=pis[:])

    nc.compile()
    return nc


def _get_program():
    global _PROGRAM
    if _PROGRAM is None:
        _PROGRAM = _build_program()
    return _PROGRAM


def run(inputs, trace=False, trace_cores=None, tmpdir=None):
    """Returns (outputs_tuple, BassKernelResults)."""
    import ml_dtypes

    from concourse.bass_utils import run_bass_kernel_spmd

    nc = _get_program()

    x = np.asarray(inputs["x"], dtype=np.float32)
    noise = np.asarray(inputs["noise"], dtype=np.float32)
    W1 = np.asarray(inputs["W1"], dtype=np.float32)
    b1 = np.asarray(inputs["b1"], dtype=np.float32)
    W2 = np.asarray(inputs["W2"], dtype=np.float32)
    b2 = np.asarray(inputs["b2"], dtype=np.float32)
    W3 = np.asarray(inputs["W3"], dtype=np.float32)
    b3 = np.asarray(inputs["b3"], dtype=np.float32)

    in_maps = []
    for e in range(E):
        in_maps.append({
            "xT": np.ascontiguousarray(x[e].T),
            "noise01": ACT_NOISE * noise[e],
            "W1": np.ascontiguousarray(W1[e]),
            "W2": np.ascontiguousarray(W2[e]),
            "W3bf": W3[e].astype(ml_dtypes.bfloat16),
            "b1c": np.ascontiguousarray(b1[e].reshape(H // P, P).T),
            "b2c": np.ascontiguousarray(b2[e].reshape(H // P, P).T),
            "b3r": b3[e].reshape(1, A),
            "ones_r": np.ones((1, P), dtype=np.float32),
        })

    res = run_bass_kernel_spmd(
        nc, in_maps, core_ids=list(range(E)), trace=trace,
        trace_cores=trace_cores, tmpdir=tmpdir,
    )
    mu = np.stack([res.results[e]["mu"] for e in range(E)])
    pi = np.stack([res.results[e]["pi"] for e in range(E)])
    return (mu, pi), res


def kernel(**inputs):
    outs, _ = run(inputs, trace=False)
    return outs
